# revision 30
# baseline (speedup 1.0000x reference)
"""HSIC pairwise loss kernel for trn2 (8 NeuronCores), fp8 DoubleRow version.

Math: with F_c = w^2 * E_c (row scaling), R the centering matrix:
    tr(R K_i R K_j) = ||G_i^T G_j||_F^2,  G_c = F_c - colmean(F_c)
and with A_ij = F_i^T F_j, s_c = F_c^T 1, u_c = F_c s_c:
    ||G_i^T G_j||^2 = ||A_ij||^2 - 2 u_i.u_j / n + ||s_i||^2 ||s_j||^2 / n^2
so only the 45 ||A_ij||_F^2 scalars need the O(n d^2) contraction; the
corrections are O(n d) and run on host in float64.

Device: inputs are host-converted to fp8e4m3 (loss error ~1.5e-3 vs the 2e-2
gate). Uniform SPMD program: every core runs the same 6-edge "claw" shape
P* = {ab, ac, bd, be, cd, ce} over 5 chunk slots; the per-core chunk->slot
mapping makes the 8x6=48 edge instances cover all 45 chunk pairs (the 3
statically-known duplicates are recomputed and subtracted on the host).
The contraction over n=4096 runs as 16 k-steps of 256 rows each
(MatmulPerfMode.DoubleRow processes 2 fp8 rows/cycle: measured 215.8 ns per
512-col matmul, i.e. the 512-cycle floor). A-blocks accumulate in one wide
6-bank PSUM tile; a single wide scalar-engine Square+accumulate ACT emits
the per-partition sum of squares (the DVE tensor_tensor_reduce path hangs
the device at runtime and is avoided); the host does the final O(1)
assembly in float64. Input tiles stream via 16 upfront DMAs round-robined
over 3 engine queues so the PE never waits mid-stream.
"""

import os as _os

import numpy as np
import ml_dtypes
from contextlib import ExitStack

import concourse.bass as bass
import concourse.tile as tile
from concourse import bacc, mybir
from concourse import bass_utils

N = 4096
EMB = 256
KP = 16              # k-steps of 256 rows (DoubleRow)
C = 5 * EMB          # 1280 data cols per k-group (5 chunk slots)
NSCAL = 8            # accum scalars per core (6 used)
WARMUP_MM = 7        # PE warmup matmuls (burn HAM cold phase during DMA)

# Shape P*: slot layout [b,c,d,e,a] with col offsets b=0, c=256, d=512,
# e=768, a=1024; claws a x {b,c}, b x {d,e}, c x {d,e}.
A_OFF = 4 * EMB

# 8 instances (a,b,c,d,e) covering all 45 chunk pairs (3 dup edges).
TUPLES = [
    (3, 1, 5, 2, 7), (6, 2, 0, 8, 7), (5, 1, 8, 4, 6), (2, 4, 3, 6, 9),
    (3, 0, 7, 9, 4), (5, 9, 0, 2, 1), (9, 6, 8, 7, 1), (6, 3, 5, 4, 8),
]


def _edges_of(t):
    a, b, c, d, e = t
    return [(a, b), (a, c), (b, d), (b, e), (c, d), (c, e)]


def _edge_mult():
    mult = {}
    for t in TUPLES:
        for e in _edges_of(t):
            key = tuple(sorted(e))
            mult[key] = mult.get(key, 0) + 1
    return mult


# units: (stat_col, mov_col); all 512-out DoubleRow matmuls
UNITS = [(A_OFF, 0), (A_OFF + 128, 0),
         (0, 512), (128, 512),
         (EMB, 512), (EMB + 128, 512)]

_CACHE = {}


def _build():
    f32 = mybir.dt.float32
    f8 = mybir.dt.float8e4
    DR = mybir.MatmulPerfMode.DoubleRow
    nc = bacc.Bacc("TRN2", target_bir_lowering=False, debug=False,
                   num_devices=8)
    x = nc.dram_tensor("x", [KP * 128, 2 * C], f8, kind="ExternalInput").ap()
    out = nc.dram_tensor("out", [128, NSCAL], f32,
                         kind="ExternalOutput").ap()

    with tile.TileContext(nc) as tc:
        with ExitStack() as ctx:
            spool = ctx.enter_context(tc.tile_pool(name="sw", bufs=1))
            xpool = ctx.enter_context(tc.tile_pool(name="xs", bufs=KP))
            psum = ctx.enter_context(tc.tile_pool(name="ps", bufs=1,
                                                  space="PSUM"))

            # one wide PSUM tile (6 banks); matmuls write bank-aligned
            # 512-col slices, the reduce reads it in one wide ACT
            pw = psum.tile([128, 3072], f32, tag="pw", name="pw")
            pslice = [(pw, 512 * i) for i in range(6)]
            p1 = pw

            # PE warmup on constant data: burns the HAM cold phase while the
            # first input tiles stream in. Overwritten by the real k=0 MMs.
            if WARMUP_MM:
                # zeroed via the scalar engine: it boots ~2us before the
                # vector engine, so the warmup matmuls start sooner
                wt = spool.tile([128, 2, 512], f8, tag="warm")
                nc.scalar.memzero(wt[:])
                for i in range(WARMUP_MM):
                    nc.tensor.matmul(p1[:, 0:512], wt[:, :, 0:128],
                                     wt[:, :, 0:512], start=True,
                                     stop=True, perf_mode=DR,
                                     skip_group_check=True)
            # scratch for reduce stage + ACT table preload
            sscr = spool.tile([128, 3072], f32, tag="sscr")
            acc = spool.tile([128, NSCAL], f32, tag="acc")
            nc.scalar.memzero(acc[:])
            nc.scalar.activation(sscr[:, 0:2], acc[:, 0:2],
                                 mybir.ActivationFunctionType.Square)

            # 16 per-k tiles; DMAs issued round-robin over 3 engine
            # queues (a single queue paces issues at ~640ns each and the
            # last tile would not even be issued until ~17us)
            qengs = [nc.sync, nc.gpsimd, nc.scalar]
            xrs = []
            for k in range(KP):
                xr = xpool.tile([128, 2, C], f8)
                qengs[k % 3].dma_start(xr[:], x[k * 128:(k + 1) * 128, :])
                xrs.append(xr)

            def mm(u, k):
                so, mo = UNITS[u]
                pt, poff = pslice[u]
                nc.tensor.matmul(
                    pt[:, poff:poff + 512],
                    xrs[k][:, :, so:so + 128],
                    xrs[k][:, :, mo:mo + 512],
                    start=(k == 0), stop=(k == KP - 1),
                    perf_mode=DR)

            # phase 1: k-outer (paced by the DMA stream)
            K1 = KP   # pure k-outer; the reduce cannot overlap anyway
            for k in range(K1):
                for u in range(6):
                    mm(u, k)
            # phase 2: unit-major (keeps PE dense; reduce fires at end)
            for u in range(6):
                for k in range(K1, KP):
                    mm(u, k)
            nc.scalar.activation(
                sscr[:, 0:3072], pw[:, 0:3072],
                mybir.ActivationFunctionType.Square,
                accum_out=acc[:, 0:1])
            nc.sync.dma_start(out, acc[:])
    nc.compile()
    return nc


def _get_nc():
    if "nc" not in _CACHE:
        _CACHE["nc"] = _build()
    return _CACHE["nc"]


def _prep_inputs(F8):
    """F8: [N, 2560] fp8 array (already scaled). Returns per-core in_maps
    with the k-pair row interleave the DoubleRow APs expect."""
    maps = []
    for t in TUPLES:
        a, b, c, d, e = t
        order = [b, c, d, e, a]
        xc = np.concatenate(
            [F8[:, u * EMB:(u + 1) * EMB] for u in order], axis=1)
        xc = np.ascontiguousarray(
            xc.reshape(KP, 2, 128, C).transpose(0, 2, 1, 3)
              .reshape(KP * 128, 2 * C))
        maps.append({"x": xc})
    return maps


def kernel(final_readout, weight, _trace=False):
    X = np.asarray(final_readout, np.float32)
    w = np.asarray(weight, np.float32)
    F64 = (w.astype(np.float64) ** 2) * X.astype(np.float64)
    F32 = F64.astype(np.float32)

    # power-of-2 scale into the fp8 sweet spot (exact to undo)
    mx = float(np.abs(F32).max())
    gamma = 2.0 ** int(np.clip(np.floor(np.log2(100.0 / mx)) if mx > 0
                               else 0, -30, 30))
    F8 = (F32 * np.float32(gamma)).astype(ml_dtypes.float8_e4m3)

    nc = _get_nc()
    if _os.environ.get("BASS_KERNEL_SIM"):
        from concourse.bass_interp import CoreSim
        results = []
        for im in _prep_inputs(F8):
            sim = CoreSim(nc, trace=False)
            sim.tensor("x")[:] = im["x"]
            sim.simulate(check_with_hw=False)
            results.append({"out": np.array(sim.tensor("out"))})
        res = bass_utils.BassKernelResults(
            results=results, instructions_and_trace=None,
            profile_json=None, exec_time_ns=None)
    else:
        res = bass_utils.run_bass_kernel_spmd(
            nc, _prep_inputs(F8), core_ids=list(range(8)), trace=_trace)
    _CACHE["last_results"] = res

    # device total of ||A8_ij||^2 over all 48 instances (scaled by gamma^4)
    acc_sum = 0.0
    for ci in range(8):
        acc_sum += float(res.results[ci]["out"].astype(np.float64).sum())

    # subtract the duplicate instances (identical fp8 data -> host fp32
    # recompute matches the device value to ~1e-7)
    F8f = F8.astype(np.float32)
    for (i, j), m in _edge_mult().items():
        if m > 1:
            a8 = F8f[:, i * EMB:(i + 1) * EMB].T @ \
                 F8f[:, j * EMB:(j + 1) * EMB]
            acc_sum -= (m - 1) * float((a8.astype(np.float64) ** 2).sum())
    total_sq = acc_sum / (gamma ** 4)

    # exact corrections in float64
    s = F64.sum(axis=0)
    loss = total_sq
    for i in range(10):
        si = s[i * EMB:(i + 1) * EMB]
        ui = F64[:, i * EMB:(i + 1) * EMB] @ si
        for j in range(i + 1, 10):
            sj = s[j * EMB:(j + 1) * EMB]
            uj = F64[:, j * EMB:(j + 1) * EMB] @ sj
            loss += -2.0 / N * float(ui @ uj) \
                + float(si @ si) * float(sj @ sj) / (N * N)
    loss /= float((N - 1) * (N - 1))
    return np.asarray([loss], np.float32)


# revision 31
# speedup vs baseline: 1.1409x; 1.1409x over previous
"""HSIC pairwise loss kernel for trn2 (8 NeuronCores), fp8 DoubleRow version.

Math: with F_c = w^2 * E_c (row scaling), R the centering matrix:
    tr(R K_i R K_j) = ||G_i^T G_j||_F^2,  G_c = F_c - colmean(F_c)
and with A_ij = F_i^T F_j, s_c = F_c^T 1, u_c = F_c s_c:
    ||G_i^T G_j||^2 = ||A_ij||^2 - 2 u_i.u_j / n + ||s_i||^2 ||s_j||^2 / n^2
so only the 45 ||A_ij||_F^2 scalars need the O(n d^2) contraction; the
corrections are O(n d) and run on host in float64.

Device: inputs are host-converted to fp8e4m3 (loss error ~1.5e-3 vs the 2e-2
gate). Uniform SPMD program: every core runs the same 6-edge "claw" shape
P* = {ab, ac, bd, be, cd, ce} over 5 chunk slots; the per-core chunk->slot
mapping makes the 8x6=48 edge instances cover all 45 chunk pairs (the 3
statically-known duplicates are recomputed and subtracted on the host).
The contraction over n=4096 runs as 16 k-steps of 256 rows each
(MatmulPerfMode.DoubleRow processes 2 fp8 rows/cycle: measured 215.8 ns per
512-col matmul, i.e. the 512-cycle floor). A-blocks accumulate in one wide
6-bank PSUM tile; a single wide scalar-engine Square+accumulate ACT emits
the per-partition sum of squares (the DVE tensor_tensor_reduce path hangs
the device at runtime and is avoided); the host does the final O(1)
assembly in float64. Input tiles stream via 16 upfront DMAs round-robined
over 3 engine queues so the PE never waits mid-stream.
"""

import os as _os

import numpy as np
import ml_dtypes
from contextlib import ExitStack

import concourse.bass as bass
import concourse.tile as tile
from concourse import bacc, mybir
from concourse import bass_utils

N = 4096
EMB = 256
KP = 16              # k-steps of 256 rows (DoubleRow)
C = 5 * EMB          # 1280 data cols per k-group (5 chunk slots)
NSCAL = 8            # accum scalars per core (6 used)
WARMUP_MM = 4        # PE warmup matmuls (burn HAM cold phase during DMA)

# Shape P*: slot layout [b,c,d,e,a] with col offsets b=0, c=256, d=512,
# e=768, a=1024; claws a x {b,c}, b x {d,e}, c x {d,e}.
A_OFF = 4 * EMB

# 8 instances (a,b,c,d,e) covering all 45 chunk pairs (3 dup edges).
TUPLES = [
    (3, 1, 5, 2, 7), (6, 2, 0, 8, 7), (5, 1, 8, 4, 6), (2, 4, 3, 6, 9),
    (3, 0, 7, 9, 4), (5, 9, 0, 2, 1), (9, 6, 8, 7, 1), (6, 3, 5, 4, 8),
]


def _edges_of(t):
    a, b, c, d, e = t
    return [(a, b), (a, c), (b, d), (b, e), (c, d), (c, e)]


def _edge_mult():
    mult = {}
    for t in TUPLES:
        for e in _edges_of(t):
            key = tuple(sorted(e))
            mult[key] = mult.get(key, 0) + 1
    return mult


# units: (stat_col, mov_col); all 512-out DoubleRow matmuls
UNITS = [(A_OFF, 0), (A_OFF + 128, 0),
         (0, 512), (128, 512),
         (EMB, 512), (EMB + 128, 512)]

_CACHE = {}


def _build():
    f32 = mybir.dt.float32
    f8 = mybir.dt.float8e4
    DR = mybir.MatmulPerfMode.DoubleRow
    nc = bacc.Bacc("TRN2", target_bir_lowering=False, debug=False,
                   num_devices=8)
    x = nc.dram_tensor("x", [KP * 128, 2 * C], f8, kind="ExternalInput").ap()
    out = nc.dram_tensor("out", [128, NSCAL], f32,
                         kind="ExternalOutput").ap()

    with tile.TileContext(nc) as tc:
        with ExitStack() as ctx:
            spool = ctx.enter_context(tc.tile_pool(name="sw", bufs=1))
            xpool = ctx.enter_context(tc.tile_pool(name="xs", bufs=KP))
            psum = ctx.enter_context(tc.tile_pool(name="ps", bufs=1,
                                                  space="PSUM"))

            # one wide PSUM tile (6 banks); matmuls write bank-aligned
            # 512-col slices, the reduce reads it in one wide ACT
            pw = psum.tile([128, 3072], f32, tag="pw", name="pw")
            pslice = [(pw, 512 * i) for i in range(6)]
            p1 = pw

            # PE warmup on constant data: burns the HAM cold phase while the
            # first input tiles stream in. Overwritten by the real k=0 MMs.
            if WARMUP_MM:
                wt = spool.tile([128, 2, 512], f8, tag="warm")
                nc.vector.memset(wt[:], 1.0)
                for i in range(WARMUP_MM):
                    nc.tensor.matmul(p1[:, 0:512], wt[:, :, 0:128],
                                     wt[:, :, 0:512], start=True,
                                     stop=True, perf_mode=DR,
                                     skip_group_check=True)
            # scratch for reduce stage + ACT table preload
            sscr = spool.tile([128, 3072], f32, tag="sscr")
            acc = spool.tile([128, NSCAL], f32, tag="acc")
            nc.vector.memset(acc[:], 0.0)
            nc.scalar.activation(sscr[:, 0:2], acc[:, 0:2],
                                 mybir.ActivationFunctionType.Square)

            # 16 per-k tiles; DMAs issued round-robin over 3 engine
            # queues (a single queue paces issues at ~640ns each and the
            # last tile would not even be issued until ~17us)
            qengs = [nc.sync, nc.gpsimd, nc.scalar]
            xrs = []
            for k in range(KP):
                xr = xpool.tile([128, 2, C], f8)
                qengs[k % 3].dma_start(xr[:], x[k * 128:(k + 1) * 128, :])
                xrs.append(xr)

            def mm(u, k):
                so, mo = UNITS[u]
                pt, poff = pslice[u]
                nc.tensor.matmul(
                    pt[:, poff:poff + 512],
                    xrs[k][:, :, so:so + 128],
                    xrs[k][:, :, mo:mo + 512],
                    start=(k == 0), stop=(k == KP - 1),
                    perf_mode=DR)

            # phase 1: k-outer (paced by the DMA stream)
            K1 = KP   # pure k-outer; the reduce cannot overlap anyway
            for k in range(K1):
                for u in range(6):
                    mm(u, k)
            # phase 2: unit-major (keeps PE dense; reduce fires at end)
            for u in range(6):
                for k in range(K1, KP):
                    mm(u, k)
            nc.scalar.activation(
                sscr[:, 0:3072], pw[:, 0:3072],
                mybir.ActivationFunctionType.Square,
                accum_out=acc[:, 0:1])
            nc.sync.dma_start(out, acc[:])
    nc.compile()
    return nc


def _get_nc():
    if "nc" not in _CACHE:
        _CACHE["nc"] = _build()
    return _CACHE["nc"]


def _prep_inputs(F8):
    """F8: [N, 2560] fp8 array (already scaled). Returns per-core in_maps
    with the k-pair row interleave the DoubleRow APs expect."""
    maps = []
    for t in TUPLES:
        a, b, c, d, e = t
        order = [b, c, d, e, a]
        xc = np.concatenate(
            [F8[:, u * EMB:(u + 1) * EMB] for u in order], axis=1)
        xc = np.ascontiguousarray(
            xc.reshape(KP, 2, 128, C).transpose(0, 2, 1, 3)
              .reshape(KP * 128, 2 * C))
        maps.append({"x": xc})
    return maps


def kernel(final_readout, weight, _trace=False):
    X = np.asarray(final_readout, np.float32)
    w = np.asarray(weight, np.float32)
    F64 = (w.astype(np.float64) ** 2) * X.astype(np.float64)
    F32 = F64.astype(np.float32)

    # power-of-2 scale into the fp8 sweet spot (exact to undo)
    mx = float(np.abs(F32).max())
    gamma = 2.0 ** int(np.clip(np.floor(np.log2(100.0 / mx)) if mx > 0
                               else 0, -30, 30))
    F8 = (F32 * np.float32(gamma)).astype(ml_dtypes.float8_e4m3)

    nc = _get_nc()
    if _os.environ.get("BASS_KERNEL_SIM"):
        from concourse.bass_interp import CoreSim
        results = []
        for im in _prep_inputs(F8):
            sim = CoreSim(nc, trace=False)
            sim.tensor("x")[:] = im["x"]
            sim.simulate(check_with_hw=False)
            results.append({"out": np.array(sim.tensor("out"))})
        res = bass_utils.BassKernelResults(
            results=results, instructions_and_trace=None,
            profile_json=None, exec_time_ns=None)
    else:
        res = bass_utils.run_bass_kernel_spmd(
            nc, _prep_inputs(F8), core_ids=list(range(8)), trace=_trace)
    _CACHE["last_results"] = res

    # device total of ||A8_ij||^2 over all 48 instances (scaled by gamma^4)
    acc_sum = 0.0
    for ci in range(8):
        acc_sum += float(res.results[ci]["out"].astype(np.float64).sum())

    # subtract the duplicate instances (identical fp8 data -> host fp32
    # recompute matches the device value to ~1e-7)
    F8f = F8.astype(np.float32)
    for (i, j), m in _edge_mult().items():
        if m > 1:
            a8 = F8f[:, i * EMB:(i + 1) * EMB].T @ \
                 F8f[:, j * EMB:(j + 1) * EMB]
            acc_sum -= (m - 1) * float((a8.astype(np.float64) ** 2).sum())
    total_sq = acc_sum / (gamma ** 4)

    # exact corrections in float64
    s = F64.sum(axis=0)
    loss = total_sq
    for i in range(10):
        si = s[i * EMB:(i + 1) * EMB]
        ui = F64[:, i * EMB:(i + 1) * EMB] @ si
        for j in range(i + 1, 10):
            sj = s[j * EMB:(j + 1) * EMB]
            uj = F64[:, j * EMB:(j + 1) * EMB] @ sj
            loss += -2.0 / N * float(ui @ uj) \
                + float(si @ si) * float(sj @ sj) / (N * N)
    loss /= float((N - 1) * (N - 1))
    return np.asarray([loss], np.float32)
